# revision 19
# baseline (speedup 1.0000x reference)
"""EnsembleMLP fused kernel for Trainium2, 8 NeuronCores (SPMD, batch-parallel).

Math transformation
-------------------
reference:
    hidden = tanh(x @ W_in.T)                                   [B, H]
    feats[b,m,e] = hidden[b, ids[m,e]]                          [B, M, E]
    preds[b,m,o] = sum_e feats[b,m,e] * W_pred[m,o,e]           [B, M, O]
    out = preds.mean(axis=1)                                    [B, O]

The gather + per-member matmul + mean are all linear in `hidden`, so they
collapse into a single [H, O] matrix:
    A[h,o] = (1/M) * sum_{(m,e): ids[m,e]==h} W_pred[m,o,e]
    out    = tanh(x @ W_in.T) @ A

A is built on the host from the tiny W_pred/ids tensors; the device does the
two matmuls + tanh. Sharding: data-parallel over batch — each of the 8 cores
takes 512 rows of x; W_in^T and A are replicated. No collectives.

Device layout (per core)
------------------------
All DRAM inputs are host-packed partition-major ([128, free]) so every DMA
moves >=1KB-contiguous per-partition segments:
  xt  [128, 4*512]   bf16  x^T slice:  xt[p, n*512+b] = x[c*512+b, n*128+p]
  wt  [128, 32*512]  bf16  W_in^T:     wt[p, t*512+n*128+h] = W_in[t*128+h, n*128+p]
  aw  [128, 32*10]   bf16  A packed:   aw[p, t*10+o] = A[t*128+p, o]
  out [10, 4*512]    f32   four ensemble column-group partials side by side;
                           host sums groups + transposes

Schedule v2 (trace-driven; L1 PE stream is the 27.6us roofline, everything
else is packed around it):
  - Head split across all 3 DMA queues in consumption order. Scalar/HWDGE
    carries xt0; Sync/HWDGE carries wt0, wt1; GpSimd/SWDGE (the fastest
    queue, ~340GB/s with big elems) carries xt1, xt2, xt3, wt2, wt3, then
    the bulk groups {4-7},{8-15},{16-23},{24-31} and aw. Keeping Sync's
    last head transfer (wt1) done by ~10.5us before SWDGE reaches the bulk
    avoids the SDMA round-robin starving the HWDGE head (the 16 SDMA
    engines round-robin between rings at packet granularity).
  - First-layer contraction chunks run in order [1,2,0,3] matching DMA
    arrival (xt1 lands first on SWDGE, xt0 last on the slower HWDGE), so
    L1 starts on wt0-land instead of xt0-land.
  - One semaphore per DMA. A cumulative per-ring counter is NOT sound: a
    DMA's 16 lane-incs are not uniformly 1-per-SDMA-engine, so >=16k can
    fire before the k-th transfer fully lands (observed as tile-0
    corruption).
  - PE warm-up (~21 matmuls on random nonzero bf16, own PSUM bank) bridges
    the DMA wait and lifts the HAM clock gate before real matmuls start.
  - First layer: 32 h-tiles x 4 i-chunks into PSUM banks t%4, tanh on ACT
    (PSUM->SBUF bf16). Ensemble: 32 matmuls 4-way column-tiled at cols
    0/32/64/96; the last four tiles land in groups 1/2/3/0 so the final
    round is a fully-overlapped quad and the stops (t28/29/30/31) are in
    distinct column groups.
  - A full-width dummy matmul into the warm-up bank replaces the old
    pe.drain() (1.3us): its streaming pushes the final ensemble MM's lazy
    PSUM writes out, then its then_inc publishes sm2. Non-final stop MMs
    are covered by successor streaming.
  - Tail: the four [10,512] f32 column-group partials are copied
    PSUM->SBUF side by side by DVE (groups 1,2,3, cross-partition) and ACT
    (group 0, same-partition, last to stop), then ONE plain [10,2048] DMA
    on Sync ships them; the host sums the four group slices and transposes.
"""

import os

import numpy as np
import ml_dtypes

BATCH, IN_DIM, HIDDEN, N_MEMBERS, ENS, OUT = 4096, 512, 4096, 256, 64, 10
NCORES = 8
B_LOC = BATCH // NCORES      # 512 batch rows per core
HT = 128                     # h-tile height (PSUM partition dim)
NHT = HIDDEN // HT           # 32 h-tiles
NIC = IN_DIM // 128          # 4 contraction chunks for the first matmul
N_WARM = 34                  # warm-up matmuls bridging DMA-wait at start
CH_ORDER = [0, 1, 2, 3]      # L1 contraction-chunk order = xt DMA arrival order

_compiled = None
LAST_RESULT = None           # BassKernelResults of the most recent run


def _build_raw():
    """Hand-scheduled Bass version (no Tile framework)."""
    from concourse import bacc, mybir

    bf16 = mybir.dt.bfloat16
    f32 = mybir.dt.float32

    nc = bacc.Bacc(
        "TRN2",
        target_bir_lowering=False,
        debug=False,
        enable_asserts=False,
        num_devices=NCORES,
    )
    xt = nc.dram_tensor("xt", [128, NIC * B_LOC], bf16, kind="ExternalInput")
    wt = nc.dram_tensor("wt", [128, NHT * NIC * HT], bf16, kind="ExternalInput")
    aw = nc.dram_tensor("aw", [128, NHT * OUT], bf16, kind="ExternalInput")
    out = nc.dram_tensor("out", [OUT, 4 * B_LOC], f32, kind="ExternalOutput")

    warm_sb = nc.alloc_sbuf_tensor("warm_sb", [128, 128], mybir.dt.uint16)
    dge_warm = nc.alloc_sbuf_tensor("dge_warm", [1, 1024], bf16)
    dummy_sb = nc.alloc_sbuf_tensor("dummy_sb", [1, 16], f32)
    xt_sb = nc.alloc_sbuf_tensor("xt_sb", [128, NIC, B_LOC], bf16)
    # wt SBUF tensors, one per DMA: head tiles 0-3, bulk groups {4-7} etc.
    wt_head = nc.alloc_sbuf_tensor("wt_head", [128, 4, NIC, HT], bf16)
    wt_bulk = [
        nc.alloc_sbuf_tensor("wt_g47", [128, 4, NIC, HT], bf16),
        nc.alloc_sbuf_tensor("wt_g8f", [128, 8, NIC, HT], bf16),
        nc.alloc_sbuf_tensor("wt_g1017", [128, 8, NIC, HT], bf16),
        nc.alloc_sbuf_tensor("wt_g181f", [128, 8, NIC, HT], bf16),
    ]
    ht_sb = [
        nc.alloc_sbuf_tensor(f"ht_sb{t}", [128, B_LOC], bf16) for t in range(NHT)
    ]
    a_sb = nc.alloc_sbuf_tensor("a_sb", [128, NHT * OUT], bf16)
    out_sb = nc.alloc_sbuf_tensor("out_sb", [OUT, 4 * B_LOC], f32)

    ps = [nc.alloc_psum_tensor(f"ps{k}", [128, B_LOC], f32) for k in range(4)]
    pso = [nc.alloc_psum_tensor(f"pso{j}", [128, B_LOC], f32) for j in range(2)]
    # Warm-up (and the final cover MM) gets its own bank: its large-magnitude
    # writes drain lazily and would race tile 0's start=True accumulation if
    # they shared ps[0].
    psw = nc.alloc_psum_tensor("psw", [128, B_LOC], f32)

    # Per-DMA completion semaphores (see module docstring).
    s_xt = nc.alloc_semaphore("s_xt")    # full xt landed
    s_wt = nc.alloc_semaphore("s_wt")    # wt head tiles 0-3 landed
    s_gd = [nc.alloc_semaphore(f"s_gd{g}") for g in range(4)]
    s_aw = nc.alloc_semaphore("s_aw")
    sm = nc.alloc_semaphore("sm")     # first-layer tile t accumulated
    sa = nc.alloc_semaphore("sa")     # tanh t done
    sm2 = nc.alloc_semaphore("sm2")   # ensemble matmul count (+1 cover MM)
    sv = nc.alloc_semaphore("sv")     # warm fill (2)
    s_cp = nc.alloc_semaphore("s_cp") # ensemble column-group copies done
    s_out = nc.alloc_semaphore("s_out")

    tanh = mybir.ActivationFunctionType.Tanh

    # ---- Warm-up tile on DVE: random bits masked to bf16 in [1, 2).
    # (HAM's activity monitor ignores all-zero matmuls.)
    if os.environ.get("KERNEL_SIMSAFE") == "1":
        fill = nc.vector.memset(warm_sb.ap(), 0x3F80)  # CoreSim xorwow workaround
    else:
        fill = nc.vector.random(warm_sb.ap())
    fill.then_inc(sv, 1)
    nc.vector.wait_ge(sv, 1)
    nc.vector.tensor_scalar(
        out=warm_sb.ap(),
        in0=warm_sb.ap(),
        scalar1=0x007F,
        scalar2=0x3F80,
        op0=mybir.AluOpType.bitwise_and,
        op1=mybir.AluOpType.bitwise_or,
    ).then_inc(sv, 1)

    # ---- DMAs, all issued immediately at user-code start, in consumption
    # order per queue.
    xt_view = xt.ap().rearrange("p (n b) -> p n b", n=NIC)
    wt_view = wt.ap().rearrange("p (t n h) -> p t n h", t=NHT, n=NIC)

    # The first transfer on each DMA queue pays a ~3us cold-start pipeline
    # latency that dominates any transfer-size effect (a 128KB first DMA
    # lands no earlier than a 512KB one: both are ~128 descriptors and the
    # pipeline is descriptor-count/latency-bound, not byte-bound). So the
    # head is two maximal-contiguity transfers, one per HWDGE queue:
    # Scalar carries ALL of xt (128 descs x 4KB), Sync carries wt tiles 0-3
    # (128 descs x 4KB); both land ~11.3us and L1 then runs stall-free
    # through tile 3. GpSimd/SWDGE absorbs its own (bigger) cold-start on a
    # 1-descriptor dummy, then waits for the head to land (the 16 SDMA
    # engines round-robin between rings at packet granularity, so
    # concurrent bulk traffic would starve the head) before streaming the
    # bulk groups + aw.
    nc.scalar.dma_start(out=xt_sb.ap(), in_=xt_view).then_inc(s_xt, 16)
    nc.sync.dma_start(out=wt_head.ap(), in_=wt_view[:, 0:4, :, :]).then_inc(s_wt, 16)
    nc.gpsimd.dma_start(out=dge_warm.ap(), in_=xt.ap()[0:1, 0:1024]).then_inc(s_out, 16)
    nc.gpsimd.wait_ge(s_wt, 16)
    nc.gpsimd.wait_ge(s_xt, 16)
    nc.gpsimd.dma_start(out=wt_bulk[0].ap(), in_=wt_view[:, 4:8, :, :]).then_inc(s_gd[0], 16)
    nc.gpsimd.dma_start(out=wt_bulk[1].ap(), in_=wt_view[:, 8:16, :, :]).then_inc(s_gd[1], 16)
    nc.gpsimd.dma_start(out=wt_bulk[2].ap(), in_=wt_view[:, 16:24, :, :]).then_inc(s_gd[2], 16)
    nc.gpsimd.dma_start(out=wt_bulk[3].ap(), in_=wt_view[:, 24:32, :, :]).then_inc(s_gd[3], 16)
    nc.gpsimd.dma_start(out=a_sb.ap(), in_=aw.ap()).then_inc(s_aw, 16)

    def tile_ref(t):
        if t < 4:
            return wt_head, t, (s_wt, 16)
        if t < 8:
            return wt_bulk[0], t - 4, (s_gd[0], 16)
        if t < 16:
            return wt_bulk[1], t - 8, (s_gd[1], 16)
        if t < 24:
            return wt_bulk[2], t - 16, (s_gd[2], 16)
        return wt_bulk[3], t - 24, (s_gd[3], 16)

    # ---- PE
    pe = nc.tensor
    pe.wait_ge(sv, 2)
    warm_bf = warm_sb.ap().bitcast(bf16)
    for _ in range(N_WARM):
        pe.matmul(
            out=psw.ap()[:, :128],
            lhsT=warm_bf,
            rhs=warm_bf,
            start=True,
            stop=True,
        )
    seen_sems = set()
    for t in range(NHT):
        wt_t, i, (sem, thresh) = tile_ref(t)
        if id(sem) not in seen_sems:
            pe.wait_ge(sem, thresh)
            seen_sems.add(id(sem))
        if t == 0:
            pe.wait_ge(s_xt, 16)          # full xt landed
        if t >= 4:
            pe.wait_ge(sa, t - 3)         # psum bank free after tanh(t-4)
        for k, n in enumerate(CH_ORDER):
            mm = pe.matmul(
                out=ps[t % 4].ap(),
                lhsT=wt_t.ap()[:, i, n, :],
                rhs=xt_sb.ap()[:, n, :],
                start=(k == 0),
                stop=(k == NIC - 1),
            )
        mm.then_inc(sm, 1)
    pe.wait_ge(s_aw, 16)                  # aw landed
    # 4-way column tiling: column group j at cols 32j, accumulating into
    # bank j%2 partitions [32j, 32j+10). Group sizes 7/8/8/9 stagger the
    # stops (t24/t28/t29/t31) so the tail copies overlap the last rounds.
    ens_group = [t % 4 for t in range(28)] + [0, 3, 0, 3]
    ens_stop = {25: 1, 26: 2, 30: 0, 31: 3}
    for t in range(NHT):
        j = ens_group[t]
        pe.wait_ge(sa, t + 1)             # ht tile t written
        pe.matmul(
            out=pso[j % 2].ap()[32 * j : 32 * j + OUT, :],
            lhsT=a_sb.ap()[:, t * OUT : (t + 1) * OUT],
            rhs=ht_sb[t].ap(),
            start=(t < 4),
            stop=(ens_stop.get(t) == j),
            tile_position=(0, 32 * j),
        ).then_inc(sm2, 1)
    # Full-width cover MM into the scratch bank: its 512-column streaming
    # pushes the final ensemble MM's lazily-drained PSUM writes out before
    # sm2 reaches 33 (cheaper than pe.drain(), ~0.2us vs ~1.3us). Non-final
    # stop MMs are covered by successor streaming.
    pe.matmul(
        out=psw.ap(),
        lhsT=warm_bf,
        rhs=ht_sb[0].ap(),
        start=True,
        stop=True,
    ).then_inc(sm2, 1)

    # ---- ACT: dummy first use pulls the act-table load off the critical
    # path; then per-tile tanh.
    act = nc.scalar
    act.wait_ge(sv, 2)
    act.activation(
        out=dummy_sb.ap(), in_=warm_sb.ap().bitcast(bf16)[:1, :16], func=tanh
    )
    for t in range(NHT):
        act.wait_ge(sm, t + 1)
        act.activation(out=ht_sb[t].ap(), in_=ps[t % 4].ap(), func=tanh).then_inc(
            sa, 1
        )

    # ---- Tail copies: column group j partial [10, 512] from PSUM partitions
    # [32j, 32j+10) to out_sb[:, 512j:512(j+1)]. Only ACT and DVE can read
    # PSUM (GpSimd cannot), and ACT cannot shift partitions, so ACT takes
    # group 0 (partitions 0-9, and the last group to stop — ACT is busy
    # with tanh until then anyway) while DVE takes groups 1-3 in stop
    # order. Wait thresholds: stop MM of group j is ensemble inc
    # #(stop_tile+1); +1 more for one successor MM's streaming to cover
    # the PSUM flush (group 0's stop is covered by the cover MM, inc #33).
    v = nc.vector
    v.wait_ge(sm2, 27)                    # grp1 stop (t=25, inc 26) + 1
    v.tensor_copy(
        out=out_sb.ap()[:, B_LOC : 2 * B_LOC], in_=pso[1].ap()[32 : 32 + OUT, :]
    ).then_inc(s_cp, 1)
    v.wait_ge(sm2, 28)                    # grp2 stop (t=26, inc 27) + 1
    v.tensor_copy(
        out=out_sb.ap()[:, 2 * B_LOC : 3 * B_LOC], in_=pso[0].ap()[64 : 64 + OUT, :]
    ).then_inc(s_cp, 1)
    v.wait_ge(sm2, 33)                    # grp3 stop (t=31, inc 32) + cover
    v.tensor_copy(
        out=out_sb.ap()[:, 3 * B_LOC : 4 * B_LOC], in_=pso[1].ap()[96 : 96 + OUT, :]
    ).then_inc(s_cp, 1)
    act.wait_ge(sm2, 32)                  # grp0 stop (t=30, inc 31) + 1
    act.activation(
        out=out_sb.ap()[:, 0:B_LOC],
        in_=pso[0].ap()[0:OUT, :],
        func=mybir.ActivationFunctionType.Copy,
    ).then_inc(s_cp, 1)

    # ---- Sync tail: single [10, 2048] out DMA once all copies landed.
    nc.sync.wait_ge(s_cp, 4)
    nc.sync.dma_start(out=out.ap(), in_=out_sb.ap()).then_inc(s_out, 16)

    nc.compile()
    return nc


def kernel(**inputs) -> np.ndarray:
    x = np.asarray(inputs["x"], dtype=np.float32)              # [4096, 512]
    W_in = np.asarray(inputs["W_in"], dtype=np.float32)        # [4096, 512]
    W_pred = np.asarray(inputs["W_pred"], dtype=np.float32)    # [256, 10, 64]
    ids = np.asarray(inputs["ensemble_input_ids"])             # [256, 64] int32

    # Collapse gather + einsum + mean into A[h, o].
    A = np.zeros((HIDDEN, OUT), dtype=np.float64)
    np.add.at(
        A,
        ids.reshape(-1),
        W_pred.transpose(0, 2, 1).reshape(-1, OUT).astype(np.float64),
    )
    A /= N_MEMBERS
    a_packed = np.ascontiguousarray(
        A.reshape(NHT, 128, OUT).transpose(1, 0, 2).reshape(128, NHT * OUT)
    ).astype(ml_dtypes.bfloat16)

    xt_bf = x.T.astype(ml_dtypes.bfloat16)                     # [512, 4096]
    wt_bf = W_in.T.astype(ml_dtypes.bfloat16)                  # [512, 4096]
    # wt packed partition-major: [p, t*512 + n*128 + h] = W_in.T[n*128+p, t*128+h]
    wt_packed = np.ascontiguousarray(
        wt_bf.reshape(NIC, 128, NHT, HT).transpose(1, 2, 0, 3).reshape(128, -1)
    )

    global _compiled
    if _compiled is None:
        _compiled = _build_raw()
    nc = _compiled

    in_maps = []
    for c in range(NCORES):
        xs = xt_bf[:, c * B_LOC : (c + 1) * B_LOC]             # [512, 512]
        xt_packed = np.ascontiguousarray(
            xs.reshape(NIC, 128, B_LOC).transpose(1, 0, 2).reshape(128, -1)
        )
        in_maps.append({"xt": xt_packed, "wt": wt_packed, "aw": a_packed})

    from concourse.bass_utils import run_bass_kernel_spmd

    trace = bool(int(os.environ.get("KERNEL_TRACE", "0")))
    res = run_bass_kernel_spmd(
        nc, in_maps, core_ids=list(range(NCORES)), trace=trace
    )
    global LAST_RESULT
    LAST_RESULT = res

    out = np.empty((BATCH, OUT), dtype=np.float32)
    for c in range(NCORES):
        o = np.asarray(res.results[c]["out"]).astype(np.float32)  # [10, 2048]
        out[c * B_LOC : (c + 1) * B_LOC, :] = (
            o[:, :B_LOC] + o[:, B_LOC : 2 * B_LOC]
            + o[:, 2 * B_LOC : 3 * B_LOC] + o[:, 3 * B_LOC :]
        ).T
    return out
